# revision 14
# baseline (speedup 1.0000x reference)
"""CoxPH loss with Efron ties on 8 trn2 NeuronCores.

Math: with per-time tables over t in [0, 2048):
    s[t] = sum_{d_i=t} exp(hr_i)
    T[t] = sum_{d_i=t, e_i=1} exp(hr_i)
    n[t] = #{d_i=t, e_i=1}
    R[t] = suffix_sum(s)[t]
the Efron correction is
    corr = sum_t sum_{k=0}^{n_t-1} log(R_t - (k/n_t) T_t)
(each rank k in 0..n_t-1 appears exactly once among the tied events, so no
sort/rank machinery is needed), and
    loss = -(sum hr*e - corr) / (sum e + 1e-7).

Device plan (SPMD on 8 cores, data-parallel over samples):
  phase 1: per-core histogram via radix one-hots over t = dlo*32 + dhi
           (dhi = d & 31 compared 32-wide, dlo = d >> 5 compared 64-wide).
           Broadcast expansions of the digit/weight streams run on ScalarE
           and GpSimd (copies), so the VectorE is_equal/mult ops are dense
           bf16 (2x mode). Accumulating bf16 PE matmuls (FWL) bin into a
           PSUM [128, 64] table holding the s/T/n quadrants.
  AllReduce of the 3x2048 tables across the 8 cores.
  phase 2: R via triangular-ones matmul suffix sum; each core selects its
           own 2 time-columns via a per-core host-provided mask input, then
           runs a masked [128,1280] log grid per column with fused
           Ln+accumulate on ScalarE.
  Output per core: [128, 3] partials (corr, hr*e, n-sum); host does the final
  tiny reduction (the unshard step).
"""

import sys

sys.path.insert(0, "/opt/trn_rl_repo")

import numpy as np

import concourse.bacc as bacc
import concourse.bass as bass
import concourse.mybir as mybir
import concourse.tile as tile

NCORES = 8
N = 4_194_304
NPC = N // NCORES            # 524288 samples per core
P = 128
CTOT = NPC // P              # 4096 free-dim columns of samples
CS = 64                      # chunk size (columns per chunk)
NCHUNK = CTOT // CS
HI = 32                      # top 5 bits of t (d >> 6)
LO = 64                      # low 6 bits of t (d & 63)
NT = 2048                    # t = dhi*64 + dlo
FT = NT // P                 # 16 columns of 128 times
KMAX = 1280                  # static bound on max ties per time (mean 1024, sd 32)
COLS_PER_CORE = FT // NCORES  # 2

F32 = mybir.dt.float32
BF16 = mybir.dt.bfloat16
I32 = mybir.dt.int32
AL = mybir.AluOpType
AF = mybir.ActivationFunctionType

_COMPILED = None


def build():
    nc = bacc.Bacc("TRN2", target_bir_lowering=False, debug=False, num_devices=NCORES)

    hr_d = nc.dram_tensor("hr", [NPC], F32, kind="ExternalInput")
    dur_d = nc.dram_tensor("dur", [NPC], I32, kind="ExternalInput")
    evt_d = nc.dram_tensor("evt", [NPC], I32, kind="ExternalInput")
    iota32_d = nc.dram_tensor("iota32", [P, HI], BF16, kind="ExternalInput")
    iota64_d = nc.dram_tensor("iota64", [P, LO], BF16, kind="ExternalInput")
    iotak_d = nc.dram_tensor("iotak", [P, KMAX], F32, kind="ExternalInput")
    tri128_d = nc.dram_tensor("tri128", [P, P], F32, kind="ExternalInput")  # [k,m]=k>=m
    tri16_d = nc.dram_tensor("tri16", [FT, FT], F32, kind="ExternalInput")  # [k,m]=k>m
    ones16_d = nc.dram_tensor("ones16", [FT, P], F32, kind="ExternalInput")
    # per-core column-selection masks: colsel[:, j*FT:(j+1)*FT] is a [P, FT]
    # 0/1 mask with a single 1-column marking this core's j-th grid column
    colsel_d = nc.dram_tensor(
        "colsel", [P, COLS_PER_CORE * FT], F32, kind="ExternalInput"
    )
    out_d = nc.dram_tensor("out", [P, 3], F32, kind="ExternalOutput")

    hr2 = hr_d.ap().rearrange("(p c) -> p c", p=P)
    dur2 = dur_d.ap().rearrange("(p c) -> p c", p=P)
    evt2 = evt_d.ap().rearrange("(p c) -> p c", p=P)

    with tile.TileContext(nc) as tc:
        with (
            tc.tile_pool(name="const", bufs=1) as constp,
            tc.tile_pool(name="data", bufs=1) as datap,
            tc.tile_pool(name="acc", bufs=1) as accp,
            tc.tile_pool(name="ps", bufs=1, space="PSUM") as psp,
            tc.tile_pool(name="dram", bufs=1, space="DRAM") as dramp,
        ):
            # ---- constants ----
            iota32 = constp.tile([P, HI], BF16)
            nc.sync.dma_start(iota32[:], iota32_d[:])
            iota64 = constp.tile([P, LO], BF16)
            nc.sync.dma_start(iota64[:], iota64_d[:])
            # dense repeated iota tiles (built once)
            iota32_x = constp.tile([P, CS, HI], BF16)
            nc.vector.tensor_copy(
                iota32_x[:], iota32[:].unsqueeze(1).broadcast_to([P, CS, HI])
            )
            iota64_x = constp.tile([P, CS, LO], BF16)
            nc.vector.tensor_copy(
                iota64_x[:], iota64[:].unsqueeze(1).broadcast_to([P, CS, LO])
            )

            # ---- full-width prep: digits, event-folded digits, weights ----
            dhi_b = datap.tile([P, CTOT], BF16)
            dlo_b = datap.tile([P, CTOT], BF16)
            dlo_e_b = datap.tile([P, CTOT], BF16)
            w_b = datap.tile([P, CTOT], BF16)
            hre_acc = accp.tile([P, 1], F32)

            with tc.tile_pool(name="prep", bufs=1) as prepp:
                hr_sb = prepp.tile([P, CTOT], F32)
                nc.sync.dma_start(hr_sb[:], hr2)
                dur_sb = prepp.tile([P, CTOT], I32)
                nc.sync.dma_start(dur_sb[:], dur2)
                evt_sb = prepp.tile([P, CTOT], I32)
                nc.sync.dma_start(evt_sb[:], evt2)

                di = prepp.tile([P, CTOT], I32, tag="di")
                di2 = prepp.tile([P, CTOT], I32, tag="di2")
                e_b = prepp.tile([P, CTOT], BF16, tag="e_b")
                e_f32 = prepp.tile([P, CTOT], F32, tag="e_f32")
                scrap_f = prepp.tile([P, CTOT], F32, tag="scrap_f")
                hre_secs = prepp.tile([P, 8], F32, tag="hre_secs")
                SEC = CTOT // 8
                for s8 in range(8):
                    ssl = slice(s8 * SEC, (s8 + 1) * SEC)
                    nc.vector.tensor_scalar(
                        di[:, ssl], dur_sb[:, ssl], 6, None, AL.logical_shift_right
                    )
                    nc.vector.tensor_copy(dhi_b[:, ssl], di[:, ssl])
                    nc.vector.tensor_scalar(
                        di2[:, ssl], dur_sb[:, ssl], 63, None, AL.bitwise_and
                    )
                    nc.vector.tensor_copy(dlo_b[:, ssl], di2[:, ssl])
                    nc.vector.tensor_copy(e_b[:, ssl], evt_sb[:, ssl])
                    # dlo_e = (dlo + 1) * e - 1   (== dlo when e=1 else -1)
                    nc.vector.scalar_tensor_tensor(
                        dlo_e_b[:, ssl], dlo_b[:, ssl], 1.0, e_b[:, ssl],
                        AL.add, AL.mult,
                    )
                    nc.vector.tensor_scalar(
                        dlo_e_b[:, ssl], dlo_e_b[:, ssl], 1.0, None, AL.subtract
                    )
                    nc.scalar.activation(w_b[:, ssl], hr_sb[:, ssl], AF.Exp)
                    nc.vector.tensor_copy(e_f32[:, ssl], evt_sb[:, ssl])
                    nc.vector.scalar_tensor_tensor(
                        scrap_f[:, ssl], hr_sb[:, ssl], 1.0, e_f32[:, ssl],
                        AL.mult, AL.mult,
                        accum_out=hre_secs[:, s8 : s8 + 1],
                    )
                nc.vector.tensor_reduce(
                    hre_acc[:], hre_secs[:], mybir.AxisListType.X, AL.add
                )

            # ---- phase 1: histogram ----
            table_ps = psp.tile([LO, P], F32)  # [w*hi|hi 64 rows] x [lo|lo_e 128 cols]
            with (
                tc.tile_pool(name="xp", bufs=2) as xpp,
                tc.tile_pool(name="oh", bufs=2) as ohp,
                tc.tile_pool(name="grid", bufs=1) as gridp,
            ):
                for ch in range(NCHUNK):
                    c0 = ch * CS
                    sl = slice(c0, c0 + CS)
                    # 64-wide expansions on ScalarE (1x; frees VectorE)
                    dlo_x = xpp.tile([P, CS, LO], BF16, tag="dlo_x")
                    nc.scalar.copy(
                        dlo_x[:], dlo_b[:, sl].unsqueeze(2).broadcast_to([P, CS, LO])
                    )
                    dlo_e_x = xpp.tile([P, CS, LO], BF16, tag="dlo_e_x")
                    nc.scalar.copy(
                        dlo_e_x[:],
                        dlo_e_b[:, sl].unsqueeze(2).broadcast_to([P, CS, LO]),
                    )

                    # one-hot builds on VectorE: lo pair dense bf16 (2x mode);
                    # eq_hi alternates dense/broadcast per chunk to balance the
                    # ScalarE expansion load against VectorE
                    lhs = ohp.tile([P, CS, P], BF16, tag="lhs")   # [0:64]=OHlo, [64:128]=OHlo_e
                    rhs = ohp.tile([P, CS, LO], BF16, tag="rhs")  # [0:32]=w*OHhi, [32:64]=OHhi
                    nc.vector.tensor_tensor(
                        lhs[:, :, 0:LO], dlo_x[:], iota64_x[:], AL.is_equal
                    )
                    nc.vector.tensor_tensor(
                        lhs[:, :, LO : 2 * LO], dlo_e_x[:], iota64_x[:], AL.is_equal
                    )
                    if ch % 2 == 0:
                        dhi_x = xpp.tile([P, CS, HI], BF16, tag="dhi_x")
                        nc.scalar.copy(
                            dhi_x[:],
                            dhi_b[:, sl].unsqueeze(2).broadcast_to([P, CS, HI]),
                        )
                        nc.vector.tensor_tensor(
                            rhs[:, :, HI : 2 * HI], dhi_x[:], iota32_x[:, :, 0:HI],
                            AL.is_equal,
                        )
                    else:
                        nc.vector.tensor_tensor(
                            rhs[:, :, HI : 2 * HI],
                            dhi_b[:, sl].unsqueeze(2).broadcast_to([P, CS, HI]),
                            iota32_x[:, :, 0:HI],
                            AL.is_equal,
                        )
                    nc.vector.tensor_tensor(
                        rhs[:, :, 0:HI],
                        rhs[:, :, HI : 2 * HI],
                        w_b[:, sl].unsqueeze(2).broadcast_to([P, CS, HI]),
                        AL.mult,
                    )
                    for c in range(CS):
                        g = ch * CS + c
                        nc.tensor.matmul(
                            table_ps[:],
                            rhs[:, c, :],
                            lhs[:, c, :],
                            start=(g == 0),
                            stop=(g == CTOT - 1),
                        )

            # table quadrants (t = hi*64 + lo):
            #   s[hi, lo] = table[0:32, 0:64]    (w*hi rows x lo cols)
            #   T[hi, lo] = table[0:32, 64:128]  (w*hi rows x lo_e cols)
            #   n[hi, lo] = table[32:64, 64:128] (hi rows x lo_e cols)
            table_sb = accp.tile([LO, P], F32)
            nc.vector.tensor_copy(table_sb[:], table_ps[:])

            ar_in = dramp.tile([3 * NT], F32)
            ar_out = dramp.tile([3 * NT], F32)
            nc.sync.dma_start(
                ar_in[:].rearrange("(a b) -> a b", a=3 * HI)[0:HI, :],
                table_sb[0:HI, 0:LO],
            )
            nc.sync.dma_start(
                ar_in[:].rearrange("(a b) -> a b", a=3 * HI)[HI : 2 * HI, :],
                table_sb[0:HI, LO:P],
            )
            nc.sync.dma_start(
                ar_in[:].rearrange("(a b) -> a b", a=3 * HI)[2 * HI : 3 * HI, :],
                table_sb[HI : 2 * HI, LO:P],
            )
            nc.gpsimd.collective_compute(
                "AllReduce",
                AL.add,
                replica_groups=[list(range(NCORES))],
                ins=[ar_in[:].opt()],
                outs=[ar_out[:].opt()],
            )

            # ---- phase 2 ----
            gridp2_cm = tc.tile_pool(name="grid2", bufs=1)
            gridp2 = gridp2_cm.__enter__()
            tri128 = constp.tile([P, P], F32)
            nc.sync.dma_start(tri128[:], tri128_d[:])
            tri16 = constp.tile([FT, FT], F32)
            nc.sync.dma_start(tri16[:], tri16_d[:])
            iotak = constp.tile([P, KMAX], F32)
            nc.sync.dma_start(iotak[:], iotak_d[:])
            ones16 = constp.tile([FT, P], F32)
            nc.sync.dma_start(ones16[:], ones16_d[:])
            colsel = constp.tile([P, COLS_PER_CORE * FT], F32)
            nc.sync.dma_start(colsel[:], colsel_d[:])

            # t = f*128 + p layouts
            s_a = accp.tile([P, FT], F32)
            nc.sync.dma_start(s_a[:], ar_out[0:NT].rearrange("(f p) -> p f", p=P))
            T_a = accp.tile([P, FT], F32)
            nc.sync.dma_start(T_a[:], ar_out[NT : 2 * NT].rearrange("(f p) -> p f", p=P))
            n_a = accp.tile([P, FT], F32)
            nc.sync.dma_start(
                n_a[:], ar_out[2 * NT : 3 * NT].rearrange("(f p) -> p f", p=P)
            )
            s_b = accp.tile([FT, P], F32)  # natural row-major [f, p] view
            nc.sync.dma_start(s_b[:], ar_out[0:NT].rearrange("(f p) -> f p", p=P))

            # R suffix sum: within-column suffix (tri128 @ s_a) plus the
            # cross-column offsets, both accumulated into one PSUM tile:
            #   offs[p, f] = sum_k ones[k, p] * (colsum[k] * [k > f])
            cs16 = accp.tile([FT, 1], F32)
            nc.vector.tensor_reduce(cs16[:], s_b[:], mybir.AxisListType.X, AL.add)
            csu = accp.tile([FT, FT], F32)
            nc.vector.tensor_scalar(csu[:], tri16[:], cs16[:, 0:1], None, AL.mult)
            rp_ps = psp.tile([P, FT], F32)
            nc.tensor.matmul(rp_ps[:], tri128[:], s_a[:], start=True, stop=False)
            nc.tensor.matmul(rp_ps[:], ones16[:], csu[:], start=False, stop=True)
            R = accp.tile([P, FT], F32)
            nc.vector.tensor_copy(R[:], rp_ps[:])

            # n is exactly integral (sums of exact 1.0s in f32); no rounding needed
            n_r = n_a
            n_s = accp.tile([P, FT], F32)
            nc.vector.tensor_scalar_max(n_s[:], n_r[:], 1.0)
            rec = accp.tile([P, FT], F32)
            nc.vector.reciprocal(rec[:], n_s[:])
            Tn = accp.tile([P, FT], F32)
            nc.vector.tensor_tensor(Tn[:], T_a[:], rec[:], AL.mult)
            negTn = accp.tile([P, FT], F32)
            nc.vector.tensor_scalar_mul(negTn[:], Tn[:], -1.0)

            nsum = accp.tile([P, 1], F32)
            nc.vector.tensor_reduce(nsum[:], n_r[:], mybir.AxisListType.X, AL.add)

            # grid over this core's columns, selected by the colsel mask:
            # my_x[j] = sum_f colsel[:, j*FT+f] * x[:, f]   (per-partition scalars)
            corr_cols = accp.tile([P, COLS_PER_CORE], F32)
            for j in range(COLS_PER_CORE):
                msl = slice(j * FT, (j + 1) * FT)
                my_negTn = accp.tile([P, 1], F32, tag="my_negTn")
                mscr = accp.tile([P, FT], F32, tag="mscr")
                nc.vector.tensor_tensor(mscr[:], negTn[:], colsel[:, msl], AL.mult)
                nc.vector.tensor_reduce(my_negTn[:], mscr[:], mybir.AxisListType.X, AL.add)
                my_R = accp.tile([P, 1], F32, tag="my_R")
                nc.vector.tensor_tensor(mscr[:], R[:], colsel[:, msl], AL.mult)
                nc.vector.tensor_reduce(my_R[:], mscr[:], mybir.AxisListType.X, AL.add)
                my_n = accp.tile([P, 1], F32, tag="my_n")
                nc.vector.tensor_tensor(mscr[:], n_r[:], colsel[:, msl], AL.mult)
                nc.vector.tensor_reduce(my_n[:], mscr[:], mybir.AxisListType.X, AL.add)

                arg = gridp2.tile([P, KMAX], F32, tag="arg")
                nc.vector.tensor_scalar(
                    arg[:], iotak[:], my_negTn[:, 0:1], my_R[:, 0:1], AL.mult, AL.add
                )
                mask = gridp2.tile([P, KMAX], F32, tag="mask")
                nc.vector.tensor_scalar(
                    mask[:], iotak[:], my_n[:, 0:1], None, AL.is_lt
                )
                margs = gridp2.tile([P, KMAX], F32, tag="margs")
                nc.vector.scalar_tensor_tensor(
                    margs[:], arg[:], 1.0, mask[:], AL.subtract, AL.mult
                )
                lscrap = gridp2.tile([P, KMAX], F32, tag="lscrap")
                nc.scalar.activation(
                    lscrap[:], margs[:], AF.Ln, bias=1.0,
                    accum_out=corr_cols[:, j : j + 1],
                )
            corr_acc = accp.tile([P, 1], F32)
            nc.vector.tensor_reduce(
                corr_acc[:], corr_cols[:], mybir.AxisListType.X, AL.add
            )

            # ---- output [128, 3] ----
            out_sb = accp.tile([P, 3], F32)
            nc.vector.tensor_copy(out_sb[:, 0:1], corr_acc[:])
            nc.vector.tensor_copy(out_sb[:, 1:2], hre_acc[:])
            nc.vector.tensor_copy(out_sb[:, 2:3], nsum[:])
            nc.sync.dma_start(out_d[:], out_sb[:])
            gridp2_cm.__exit__(None, None, None)

    nc.compile()
    return nc


def _consts():
    iota32 = np.tile(np.arange(HI), (P, 1)).astype(np.float32)
    iota64 = np.tile(np.arange(LO), (P, 1)).astype(np.float32)
    iotak = np.tile(np.arange(KMAX, dtype=np.float32), (P, 1))
    k = np.arange(P)
    tri128 = (k[:, None] >= k[None, :]).astype(np.float32)
    kf = np.arange(FT)
    tri16 = (kf[:, None] > kf[None, :]).astype(np.float32)
    return iota32, iota64, iotak, tri128, tri16


def kernel(hazard_ratio, durations, events):
    global _COMPILED
    import ml_dtypes
    from concourse.bass_utils import run_bass_kernel_spmd

    if _COMPILED is None:
        _COMPILED = build()
    nc = _COMPILED

    iota32, iota64, iotak, tri128, tri16 = _consts()
    iota32 = iota32.astype(ml_dtypes.bfloat16)
    iota64 = iota64.astype(ml_dtypes.bfloat16)
    ones16 = np.ones((FT, P), dtype=np.float32)
    hr = np.ascontiguousarray(np.asarray(hazard_ratio, dtype=np.float32).reshape(-1))
    dur = np.ascontiguousarray(np.asarray(durations, dtype=np.int32).reshape(-1))
    evt = np.ascontiguousarray(np.asarray(events, dtype=np.int32).reshape(-1))

    in_maps = []
    for c in range(NCORES):
        sl = slice(c * NPC, (c + 1) * NPC)
        colsel = np.zeros((P, COLS_PER_CORE * FT), dtype=np.float32)
        for j in range(COLS_PER_CORE):
            colsel[:, j * FT + (c * COLS_PER_CORE + j)] = 1.0
        in_maps.append(
            {
                "hr": hr[sl],
                "dur": dur[sl],
                "evt": evt[sl],
                "iota32": iota32,
                "iota64": iota64,
                "iotak": iotak,
                "tri128": tri128,
                "tri16": tri16,
                "ones16": ones16,
                "colsel": colsel,
            }
        )
    res = run_bass_kernel_spmd(nc, in_maps, list(range(NCORES)))

    outs = [res.results[c]["out"] for c in range(NCORES)]
    corr = np.float32(sum(o[:, 0].sum(dtype=np.float32) for o in outs))
    hre = np.float32(sum(o[:, 1].sum(dtype=np.float32) for o in outs))
    esum = outs[0][:, 2].sum(dtype=np.float32)
    loss = -(hre - corr) / (esum + np.float32(1e-7))
    return np.float32(loss).reshape(())


# revision 15
# speedup vs baseline: 1.0207x; 1.0207x over previous
"""CoxPH loss with Efron ties on 8 trn2 NeuronCores.

Math: with per-time tables over t in [0, 2048):
    s[t] = sum_{d_i=t} exp(hr_i)
    T[t] = sum_{d_i=t, e_i=1} exp(hr_i)
    n[t] = #{d_i=t, e_i=1}
    R[t] = suffix_sum(s)[t]
the Efron correction is
    corr = sum_t sum_{k=0}^{n_t-1} log(R_t - (k/n_t) T_t)
(each rank k in 0..n_t-1 appears exactly once among the tied events, so no
sort/rank machinery is needed), and
    loss = -(sum hr*e - corr) / (sum e + 1e-7).

Device plan (SPMD on 8 cores, data-parallel over samples):
  phase 1: per-core histogram via radix one-hots over t = dlo*32 + dhi
           (dhi = d & 31 compared 32-wide, dlo = d >> 5 compared 64-wide).
           Broadcast expansions of the digit/weight streams run on ScalarE
           and GpSimd (copies), so the VectorE is_equal/mult ops are dense
           bf16 (2x mode). Accumulating bf16 PE matmuls (FWL) bin into a
           PSUM [128, 64] table holding the s/T/n quadrants.
  AllReduce of the 3x2048 tables across the 8 cores.
  phase 2: R via triangular-ones matmul suffix sum; each core selects its
           own 2 time-columns via a per-core host-provided mask input, then
           runs a masked [128,1280] log grid per column with fused
           Ln+accumulate on ScalarE.
  Output per core: [128, 3] partials (corr, hr*e, n-sum); host does the final
  tiny reduction (the unshard step).
"""

import sys

sys.path.insert(0, "/opt/trn_rl_repo")

import numpy as np

import concourse.bacc as bacc
import concourse.bass as bass
import concourse.mybir as mybir
import concourse.tile as tile

NCORES = 8
N = 4_194_304
NPC = N // NCORES            # 524288 samples per core
P = 128
CTOT = NPC // P              # 4096 free-dim columns of samples
CS = 64                      # chunk size (columns per chunk)
NCHUNK = CTOT // CS
HI = 32                      # top 5 bits of t (d >> 6)
LO = 64                      # low 6 bits of t (d & 63)
NT = 2048                    # t = dhi*64 + dlo
FT = NT // P                 # 16 columns of 128 times
KMAX = 1280                  # static bound on max ties per time (mean 1024, sd 32)
COLS_PER_CORE = FT // NCORES  # 2

F32 = mybir.dt.float32
BF16 = mybir.dt.bfloat16
I32 = mybir.dt.int32
AL = mybir.AluOpType
AF = mybir.ActivationFunctionType

_COMPILED = None


def build():
    nc = bacc.Bacc("TRN2", target_bir_lowering=False, debug=False, num_devices=NCORES)

    hr_d = nc.dram_tensor("hr", [NPC], F32, kind="ExternalInput")
    dur_d = nc.dram_tensor("dur", [NPC], I32, kind="ExternalInput")
    evt_d = nc.dram_tensor("evt", [NPC], I32, kind="ExternalInput")
    iota32_d = nc.dram_tensor("iota32", [P, HI], BF16, kind="ExternalInput")
    iota64_d = nc.dram_tensor("iota64", [P, LO], BF16, kind="ExternalInput")
    iotak_d = nc.dram_tensor("iotak", [P, KMAX], F32, kind="ExternalInput")
    tri128_d = nc.dram_tensor("tri128", [P, P], F32, kind="ExternalInput")  # [k,m]=k>=m
    tri16_d = nc.dram_tensor("tri16", [FT, FT], F32, kind="ExternalInput")  # [k,m]=k>m
    ones16_d = nc.dram_tensor("ones16", [FT, P], F32, kind="ExternalInput")
    # per-core column-selection masks: colsel[:, j*FT:(j+1)*FT] is a [P, FT]
    # 0/1 mask with a single 1-column marking this core's j-th grid column
    colsel_d = nc.dram_tensor(
        "colsel", [P, COLS_PER_CORE * FT], F32, kind="ExternalInput"
    )
    out_d = nc.dram_tensor("out", [P, 3], F32, kind="ExternalOutput")

    hr2 = hr_d.ap().rearrange("(p c) -> p c", p=P)
    dur2 = dur_d.ap().rearrange("(p c) -> p c", p=P)
    evt2 = evt_d.ap().rearrange("(p c) -> p c", p=P)

    with tile.TileContext(nc) as tc:
        with (
            tc.tile_pool(name="const", bufs=1) as constp,
            tc.tile_pool(name="data", bufs=1) as datap,
            tc.tile_pool(name="acc", bufs=1) as accp,
            tc.tile_pool(name="ps", bufs=1, space="PSUM") as psp,
            tc.tile_pool(name="dram", bufs=1, space="DRAM") as dramp,
        ):
            # ---- constants ----
            iota32 = constp.tile([P, HI], BF16)
            nc.sync.dma_start(iota32[:], iota32_d[:])
            iota64 = constp.tile([P, LO], BF16)
            nc.sync.dma_start(iota64[:], iota64_d[:])
            # dense repeated iota tiles (built once)
            iota32_x = constp.tile([P, CS, HI], BF16)
            nc.vector.tensor_copy(
                iota32_x[:], iota32[:].unsqueeze(1).broadcast_to([P, CS, HI])
            )
            iota64_x = constp.tile([P, CS, LO], BF16)
            nc.vector.tensor_copy(
                iota64_x[:], iota64[:].unsqueeze(1).broadcast_to([P, CS, LO])
            )

            # ---- full-width prep: digits, event-folded digits, weights ----
            dhi_b = datap.tile([P, CTOT], BF16)
            dlo_b = datap.tile([P, CTOT], BF16)
            dlo_e_b = datap.tile([P, CTOT], BF16)
            w_b = datap.tile([P, CTOT], BF16)
            hre_acc = accp.tile([P, 1], F32)

            with tc.tile_pool(name="prep", bufs=1) as prepp:
                hr_sb = prepp.tile([P, CTOT], F32)
                dur_sb = prepp.tile([P, CTOT], I32)
                evt_sb = prepp.tile([P, CTOT], I32)

                di = prepp.tile([P, CTOT], I32, tag="di")
                di2 = prepp.tile([P, CTOT], I32, tag="di2")
                e_b = prepp.tile([P, CTOT], BF16, tag="e_b")
                e_f32 = prepp.tile([P, CTOT], F32, tag="e_f32")
                scrap_f = prepp.tile([P, CTOT], F32, tag="scrap_f")
                hre_secs = prepp.tile([P, 8], F32, tag="hre_secs")
                SEC = CTOT // 8
                for s8 in range(8):
                    ssl = slice(s8 * SEC, (s8 + 1) * SEC)
                    nc.sync.dma_start(dur_sb[:, ssl], dur2[:, ssl])
                    nc.sync.dma_start(hr_sb[:, ssl], hr2[:, ssl])
                    nc.sync.dma_start(evt_sb[:, ssl], evt2[:, ssl])
                    nc.vector.tensor_scalar(
                        di[:, ssl], dur_sb[:, ssl], 6, None, AL.logical_shift_right
                    )
                    nc.vector.tensor_copy(dhi_b[:, ssl], di[:, ssl])
                    nc.vector.tensor_scalar(
                        di2[:, ssl], dur_sb[:, ssl], 63, None, AL.bitwise_and
                    )
                    nc.vector.tensor_copy(dlo_b[:, ssl], di2[:, ssl])
                    nc.vector.tensor_copy(e_b[:, ssl], evt_sb[:, ssl])
                    # dlo_e = (dlo + 1) * e - 1   (== dlo when e=1 else -1)
                    nc.vector.scalar_tensor_tensor(
                        dlo_e_b[:, ssl], dlo_b[:, ssl], 1.0, e_b[:, ssl],
                        AL.add, AL.mult,
                    )
                    nc.vector.tensor_scalar(
                        dlo_e_b[:, ssl], dlo_e_b[:, ssl], 1.0, None, AL.subtract
                    )
                    nc.scalar.activation(w_b[:, ssl], hr_sb[:, ssl], AF.Exp)
                    nc.vector.tensor_copy(e_f32[:, ssl], evt_sb[:, ssl])
                    nc.vector.scalar_tensor_tensor(
                        scrap_f[:, ssl], hr_sb[:, ssl], 1.0, e_f32[:, ssl],
                        AL.mult, AL.mult,
                        accum_out=hre_secs[:, s8 : s8 + 1],
                    )
                nc.vector.tensor_reduce(
                    hre_acc[:], hre_secs[:], mybir.AxisListType.X, AL.add
                )

            # ---- phase 1: histogram ----
            table_ps = psp.tile([LO, P], F32)  # [w*hi|hi 64 rows] x [lo|lo_e 128 cols]
            with (
                tc.tile_pool(name="xp", bufs=2) as xpp,
                tc.tile_pool(name="oh", bufs=2) as ohp,
                tc.tile_pool(name="grid", bufs=1) as gridp,
            ):
                for ch in range(NCHUNK):
                    c0 = ch * CS
                    sl = slice(c0, c0 + CS)
                    # 64-wide expansions on ScalarE (1x; frees VectorE)
                    dlo_x = xpp.tile([P, CS, LO], BF16, tag="dlo_x")
                    nc.scalar.copy(
                        dlo_x[:], dlo_b[:, sl].unsqueeze(2).broadcast_to([P, CS, LO])
                    )
                    dlo_e_x = xpp.tile([P, CS, LO], BF16, tag="dlo_e_x")
                    nc.scalar.copy(
                        dlo_e_x[:],
                        dlo_e_b[:, sl].unsqueeze(2).broadcast_to([P, CS, LO]),
                    )

                    # one-hot builds on VectorE: lo pair dense bf16 (2x mode);
                    # eq_hi alternates dense/broadcast per chunk to balance the
                    # ScalarE expansion load against VectorE
                    lhs = ohp.tile([P, CS, P], BF16, tag="lhs")   # [0:64]=OHlo, [64:128]=OHlo_e
                    rhs = ohp.tile([P, CS, LO], BF16, tag="rhs")  # [0:32]=w*OHhi, [32:64]=OHhi
                    nc.vector.tensor_tensor(
                        lhs[:, :, 0:LO], dlo_x[:], iota64_x[:], AL.is_equal
                    )
                    nc.vector.tensor_tensor(
                        lhs[:, :, LO : 2 * LO], dlo_e_x[:], iota64_x[:], AL.is_equal
                    )
                    if ch % 2 == 0:
                        dhi_x = xpp.tile([P, CS, HI], BF16, tag="dhi_x")
                        nc.scalar.copy(
                            dhi_x[:],
                            dhi_b[:, sl].unsqueeze(2).broadcast_to([P, CS, HI]),
                        )
                        nc.vector.tensor_tensor(
                            rhs[:, :, HI : 2 * HI], dhi_x[:], iota32_x[:, :, 0:HI],
                            AL.is_equal,
                        )
                    else:
                        nc.vector.tensor_tensor(
                            rhs[:, :, HI : 2 * HI],
                            dhi_b[:, sl].unsqueeze(2).broadcast_to([P, CS, HI]),
                            iota32_x[:, :, 0:HI],
                            AL.is_equal,
                        )
                    nc.vector.tensor_tensor(
                        rhs[:, :, 0:HI],
                        rhs[:, :, HI : 2 * HI],
                        w_b[:, sl].unsqueeze(2).broadcast_to([P, CS, HI]),
                        AL.mult,
                    )
                    for c in range(CS):
                        g = ch * CS + c
                        nc.tensor.matmul(
                            table_ps[:],
                            rhs[:, c, :],
                            lhs[:, c, :],
                            start=(g == 0),
                            stop=(g == CTOT - 1),
                        )

            # table quadrants (t = hi*64 + lo):
            #   s[hi, lo] = table[0:32, 0:64]    (w*hi rows x lo cols)
            #   T[hi, lo] = table[0:32, 64:128]  (w*hi rows x lo_e cols)
            #   n[hi, lo] = table[32:64, 64:128] (hi rows x lo_e cols)
            table_sb = accp.tile([LO, P], F32)
            nc.vector.tensor_copy(table_sb[:], table_ps[:])

            ar_in = dramp.tile([3 * NT], F32)
            ar_out = dramp.tile([3 * NT], F32)
            nc.sync.dma_start(
                ar_in[:].rearrange("(a b) -> a b", a=3 * HI)[0:HI, :],
                table_sb[0:HI, 0:LO],
            )
            nc.sync.dma_start(
                ar_in[:].rearrange("(a b) -> a b", a=3 * HI)[HI : 2 * HI, :],
                table_sb[0:HI, LO:P],
            )
            nc.sync.dma_start(
                ar_in[:].rearrange("(a b) -> a b", a=3 * HI)[2 * HI : 3 * HI, :],
                table_sb[HI : 2 * HI, LO:P],
            )
            nc.gpsimd.collective_compute(
                "AllReduce",
                AL.add,
                replica_groups=[list(range(NCORES))],
                ins=[ar_in[:].opt()],
                outs=[ar_out[:].opt()],
            )

            # ---- phase 2 ----
            gridp2_cm = tc.tile_pool(name="grid2", bufs=1)
            gridp2 = gridp2_cm.__enter__()
            tri128 = constp.tile([P, P], F32)
            nc.sync.dma_start(tri128[:], tri128_d[:])
            tri16 = constp.tile([FT, FT], F32)
            nc.sync.dma_start(tri16[:], tri16_d[:])
            iotak = constp.tile([P, KMAX], F32)
            nc.sync.dma_start(iotak[:], iotak_d[:])
            ones16 = constp.tile([FT, P], F32)
            nc.sync.dma_start(ones16[:], ones16_d[:])
            colsel = constp.tile([P, COLS_PER_CORE * FT], F32)
            nc.sync.dma_start(colsel[:], colsel_d[:])

            # t = f*128 + p layouts
            s_a = accp.tile([P, FT], F32)
            nc.sync.dma_start(s_a[:], ar_out[0:NT].rearrange("(f p) -> p f", p=P))
            T_a = accp.tile([P, FT], F32)
            nc.sync.dma_start(T_a[:], ar_out[NT : 2 * NT].rearrange("(f p) -> p f", p=P))
            n_a = accp.tile([P, FT], F32)
            nc.sync.dma_start(
                n_a[:], ar_out[2 * NT : 3 * NT].rearrange("(f p) -> p f", p=P)
            )
            s_b = accp.tile([FT, P], F32)  # natural row-major [f, p] view
            nc.sync.dma_start(s_b[:], ar_out[0:NT].rearrange("(f p) -> f p", p=P))

            # R suffix sum: within-column suffix (tri128 @ s_a) plus the
            # cross-column offsets, both accumulated into one PSUM tile:
            #   offs[p, f] = sum_k ones[k, p] * (colsum[k] * [k > f])
            cs16 = accp.tile([FT, 1], F32)
            nc.vector.tensor_reduce(cs16[:], s_b[:], mybir.AxisListType.X, AL.add)
            csu = accp.tile([FT, FT], F32)
            nc.vector.tensor_scalar(csu[:], tri16[:], cs16[:, 0:1], None, AL.mult)
            rp_ps = psp.tile([P, FT], F32)
            nc.tensor.matmul(rp_ps[:], tri128[:], s_a[:], start=True, stop=False)
            nc.tensor.matmul(rp_ps[:], ones16[:], csu[:], start=False, stop=True)
            R = accp.tile([P, FT], F32)
            nc.vector.tensor_copy(R[:], rp_ps[:])

            # n is exactly integral (sums of exact 1.0s in f32); no rounding needed
            n_r = n_a
            n_s = accp.tile([P, FT], F32)
            nc.vector.tensor_scalar_max(n_s[:], n_r[:], 1.0)
            rec = accp.tile([P, FT], F32)
            nc.vector.reciprocal(rec[:], n_s[:])
            Tn = accp.tile([P, FT], F32)
            nc.vector.tensor_tensor(Tn[:], T_a[:], rec[:], AL.mult)
            negTn = accp.tile([P, FT], F32)
            nc.vector.tensor_scalar_mul(negTn[:], Tn[:], -1.0)

            nsum = accp.tile([P, 1], F32)
            nc.vector.tensor_reduce(nsum[:], n_r[:], mybir.AxisListType.X, AL.add)

            # grid over this core's columns, selected by the colsel mask:
            # my_x[j] = sum_f colsel[:, j*FT+f] * x[:, f]   (per-partition scalars)
            corr_cols = accp.tile([P, COLS_PER_CORE], F32)
            for j in range(COLS_PER_CORE):
                msl = slice(j * FT, (j + 1) * FT)
                my_negTn = accp.tile([P, 1], F32, tag="my_negTn")
                mscr = accp.tile([P, FT], F32, tag="mscr")
                nc.vector.tensor_tensor(mscr[:], negTn[:], colsel[:, msl], AL.mult)
                nc.vector.tensor_reduce(my_negTn[:], mscr[:], mybir.AxisListType.X, AL.add)
                my_R = accp.tile([P, 1], F32, tag="my_R")
                nc.vector.tensor_tensor(mscr[:], R[:], colsel[:, msl], AL.mult)
                nc.vector.tensor_reduce(my_R[:], mscr[:], mybir.AxisListType.X, AL.add)
                my_n = accp.tile([P, 1], F32, tag="my_n")
                nc.vector.tensor_tensor(mscr[:], n_r[:], colsel[:, msl], AL.mult)
                nc.vector.tensor_reduce(my_n[:], mscr[:], mybir.AxisListType.X, AL.add)

                arg = gridp2.tile([P, KMAX], F32, tag="arg")
                nc.vector.tensor_scalar(
                    arg[:], iotak[:], my_negTn[:, 0:1], my_R[:, 0:1], AL.mult, AL.add
                )
                mask = gridp2.tile([P, KMAX], F32, tag="mask")
                nc.vector.tensor_scalar(
                    mask[:], iotak[:], my_n[:, 0:1], None, AL.is_lt
                )
                margs = gridp2.tile([P, KMAX], F32, tag="margs")
                nc.vector.scalar_tensor_tensor(
                    margs[:], arg[:], 1.0, mask[:], AL.subtract, AL.mult
                )
                lscrap = gridp2.tile([P, KMAX], F32, tag="lscrap")
                nc.scalar.activation(
                    lscrap[:], margs[:], AF.Ln, bias=1.0,
                    accum_out=corr_cols[:, j : j + 1],
                )
            corr_acc = accp.tile([P, 1], F32)
            nc.vector.tensor_reduce(
                corr_acc[:], corr_cols[:], mybir.AxisListType.X, AL.add
            )

            # ---- output [128, 3] ----
            out_sb = accp.tile([P, 3], F32)
            nc.vector.tensor_copy(out_sb[:, 0:1], corr_acc[:])
            nc.vector.tensor_copy(out_sb[:, 1:2], hre_acc[:])
            nc.vector.tensor_copy(out_sb[:, 2:3], nsum[:])
            nc.sync.dma_start(out_d[:], out_sb[:])
            gridp2_cm.__exit__(None, None, None)

    nc.compile()
    return nc


def _consts():
    iota32 = np.tile(np.arange(HI), (P, 1)).astype(np.float32)
    iota64 = np.tile(np.arange(LO), (P, 1)).astype(np.float32)
    iotak = np.tile(np.arange(KMAX, dtype=np.float32), (P, 1))
    k = np.arange(P)
    tri128 = (k[:, None] >= k[None, :]).astype(np.float32)
    kf = np.arange(FT)
    tri16 = (kf[:, None] > kf[None, :]).astype(np.float32)
    return iota32, iota64, iotak, tri128, tri16


def kernel(hazard_ratio, durations, events):
    global _COMPILED
    import ml_dtypes
    from concourse.bass_utils import run_bass_kernel_spmd

    if _COMPILED is None:
        _COMPILED = build()
    nc = _COMPILED

    iota32, iota64, iotak, tri128, tri16 = _consts()
    iota32 = iota32.astype(ml_dtypes.bfloat16)
    iota64 = iota64.astype(ml_dtypes.bfloat16)
    ones16 = np.ones((FT, P), dtype=np.float32)
    hr = np.ascontiguousarray(np.asarray(hazard_ratio, dtype=np.float32).reshape(-1))
    dur = np.ascontiguousarray(np.asarray(durations, dtype=np.int32).reshape(-1))
    evt = np.ascontiguousarray(np.asarray(events, dtype=np.int32).reshape(-1))

    in_maps = []
    for c in range(NCORES):
        sl = slice(c * NPC, (c + 1) * NPC)
        colsel = np.zeros((P, COLS_PER_CORE * FT), dtype=np.float32)
        for j in range(COLS_PER_CORE):
            colsel[:, j * FT + (c * COLS_PER_CORE + j)] = 1.0
        in_maps.append(
            {
                "hr": hr[sl],
                "dur": dur[sl],
                "evt": evt[sl],
                "iota32": iota32,
                "iota64": iota64,
                "iotak": iotak,
                "tri128": tri128,
                "tri16": tri16,
                "ones16": ones16,
                "colsel": colsel,
            }
        )
    res = run_bass_kernel_spmd(nc, in_maps, list(range(NCORES)))

    outs = [res.results[c]["out"] for c in range(NCORES)]
    corr = np.float32(sum(o[:, 0].sum(dtype=np.float32) for o in outs))
    hre = np.float32(sum(o[:, 1].sum(dtype=np.float32) for o in outs))
    esum = outs[0][:, 2].sum(dtype=np.float32)
    loss = -(hre - corr) / (esum + np.float32(1e-7))
    return np.float32(loss).reshape(())
